# revision 16
# baseline (speedup 1.0000x reference)
import numpy as np

import concourse.bacc as bacc
import concourse.bass as bass
import concourse.mybir as mybir
from concourse.bass_utils import run_bass_kernel_spmd
from concourse.tile import TileContext

N_CORES = 8
Q, UNITS, D = 2048, 512, 128
D2 = 2 * D
QS = Q // N_CORES
UT = UNITS // 128

DT_NAME = "fp32"
G = 16
GP_SUBS = (3,)
USE_TREE = True
MINW_DVE = 16


def _dt():
    return mybir.dt.float16 if DT_NAME == "fp16" else mybir.dt.float32


def _np_dt():
    return np.float16 if DT_NAME == "fp16" else np.float32


def build_nc():
    dt = _dt()
    f32 = mybir.dt.float32
    nc = bacc.Bacc("TRN2", target_bir_lowering=False)
    x2 = nc.dram_tensor("x2", [QS, D2], dt, kind="ExternalInput")
    w2 = nc.dram_tensor("w2", [UNITS, D2], dt, kind="ExternalInput")
    out = nc.dram_tensor("out", [UT, 128, QS], f32, kind="ExternalOutput")

    n_chunks = QS // G

    with TileContext(nc) as tc:
        with (
            tc.tile_pool(name="wpool", bufs=1) as wpool,
            tc.tile_pool(name="xpool", bufs=2) as xpool,
            tc.tile_pool(name="dpool", bufs=2) as dpool,
            tc.tile_pool(name="opool", bufs=1) as opool,
        ):
            w2sb = wpool.tile([128, UT, D2], dt)
            nc.sync.dma_start(w2sb[:, :, :], w2.rearrange("(t p) d -> p t d", p=128))
            w2c = wpool.tile([128, UT, D2], dt)
            nc.vector.tensor_copy(w2c[:, :, :], w2sb[:, :, :])
            use_gp = any(k > 0 for k in GP_SUBS)
            if use_gp:
                w2g = wpool.tile([128, UT, D2], dt)
                nc.gpsimd.tensor_copy(w2g[:, :, :], w2sb[:, :, :])

            osb = opool.tile([128, UT, QS], f32)

            for c in range(n_chunks):
                k_gp = GP_SUBS[c % len(GP_SUBS)]
                td = UT - k_gp

                x2b = xpool.tile([128, G * D2], dt, tag="x2b")
                src = (
                    x2[c * G : (c + 1) * G, :]
                    .rearrange("g d -> (g d)")
                    .unsqueeze(0)
                    .broadcast_to([128, G * D2])
                )
                nc.sync.dma_start(x2b[:, :], src)
                x2b4 = x2b.rearrange("p (g d) -> p g d", d=D2).unsqueeze(1)

                diff = dpool.tile([128, UT, G, D2], dt, tag="diff", name="diff")
                if td > 0:
                    nc.vector.tensor_tensor(
                        diff[:, 0:td, :, :],
                        x2b4.broadcast_to([128, td, G, D2]),
                        w2c[:, 0:td, :].unsqueeze(2).broadcast_to([128, td, G, D2]),
                        mybir.AluOpType.subtract,
                    )
                if k_gp > 0:
                    nc.gpsimd.tensor_tensor(
                        diff[:, td:UT, :, :],
                        x2b4.broadcast_to([128, k_gp, G, D2]),
                        w2g[:, td:UT, :].unsqueeze(2).broadcast_to(
                            [128, k_gp, G, D2]
                        ),
                        mybir.AluOpType.subtract,
                    )

                width = D2
                osl = osb[:, :, c * G : (c + 1) * G]
                if USE_TREE:
                    while width > MINW_DVE and width % 2 == 0:
                        half = width // 2
                        nc.vector.tensor_tensor(
                            diff[:, :, :, 0:half],
                            diff[:, :, :, 0:half],
                            diff[:, :, :, half:width],
                            mybir.AluOpType.min,
                        )
                        width = half
                nc.vector.tensor_reduce(
                    osl,
                    diff[:, :, :, 0:width],
                    axis=mybir.AxisListType.X,
                    op=mybir.AluOpType.min,
                )

            for t in range(UT):
                nc.sync.dma_start(out[t, :, :], osb[:, t, :])

    nc.compile()
    return nc


def _prep_inputs(x, Wmin, Wmax):
    ndt = _np_dt()
    x2 = np.concatenate([x, -x], axis=1).astype(ndt)
    w2 = np.concatenate([Wmin, -Wmax], axis=1).astype(ndt)
    in_maps = []
    for r in range(N_CORES):
        in_maps.append(
            {
                "x2": np.ascontiguousarray(x2[r * QS : (r + 1) * QS]),
                "w2": np.ascontiguousarray(w2),
            }
        )
    return in_maps


def _assemble(results):
    ys = []
    for r in range(N_CORES):
        o = results[r]["out"]
        ys.append(o.reshape(UNITS, QS).T)
    return np.ascontiguousarray(np.concatenate(ys, axis=0).astype(np.float32))


_NC_CACHE = {}


def _get_nc():
    key = (DT_NAME, G, GP_SUBS, USE_TREE, MINW_DVE)
    if key not in _NC_CACHE:
        _NC_CACHE[key] = build_nc()
    return _NC_CACHE[key]


def run(x, Wmin, Wmax, trace=False):
    nc = _get_nc()
    in_maps = _prep_inputs(x, Wmin, Wmax)
    res = run_bass_kernel_spmd(nc, in_maps, core_ids=list(range(N_CORES)), trace=trace)
    return _assemble(res.results), res


def kernel(x, Wmin, Wmax):
    y, _ = run(x, Wmin, Wmax, trace=False)
    return y


# revision 17
# speedup vs baseline: 2.5292x; 2.5292x over previous
import numpy as np

import concourse.bacc as bacc
import concourse.bass as bass
import concourse.mybir as mybir
from concourse.bass_utils import run_bass_kernel_spmd
from concourse.tile import TileContext

N_CORES = 8
Q, UNITS, D = 2048, 512, 128
D2 = 2 * D
QS = Q // N_CORES
UT = UNITS // 128

DT_NAME = "fp16"
G = 16
GP_SUBS = (0,)
USE_TREE = True
MINW_DVE = 16


def _dt():
    return mybir.dt.float16 if DT_NAME == "fp16" else mybir.dt.float32


def _np_dt():
    return np.float16 if DT_NAME == "fp16" else np.float32


def build_nc():
    dt = _dt()
    f32 = mybir.dt.float32
    nc = bacc.Bacc("TRN2", target_bir_lowering=False)
    x2 = nc.dram_tensor("x2", [QS, D2], dt, kind="ExternalInput")
    w2 = nc.dram_tensor("w2", [UNITS, D2], dt, kind="ExternalInput")
    out = nc.dram_tensor("out", [UT, 128, QS], f32, kind="ExternalOutput")

    n_chunks = QS // G

    with TileContext(nc) as tc:
        with (
            tc.tile_pool(name="wpool", bufs=1) as wpool,
            tc.tile_pool(name="xpool", bufs=2) as xpool,
            tc.tile_pool(name="dpool", bufs=2) as dpool,
            tc.tile_pool(name="opool", bufs=1) as opool,
        ):
            w2sb = wpool.tile([128, UT, D2], dt)
            nc.sync.dma_start(w2sb[:, :, :], w2.rearrange("(t p) d -> p t d", p=128))
            w2c = wpool.tile([128, UT, D2], dt)
            nc.vector.tensor_copy(w2c[:, :, :], w2sb[:, :, :])
            use_gp = any(k > 0 for k in GP_SUBS)
            if use_gp:
                w2g = wpool.tile([128, UT, D2], dt)
                nc.gpsimd.tensor_copy(w2g[:, :, :], w2sb[:, :, :])

            osb = opool.tile([128, UT, QS], f32)

            for c in range(n_chunks):
                k_gp = GP_SUBS[c % len(GP_SUBS)]
                td = UT - k_gp

                x2b = xpool.tile([128, G * D2], dt, tag="x2b")
                src = (
                    x2[c * G : (c + 1) * G, :]
                    .rearrange("g d -> (g d)")
                    .unsqueeze(0)
                    .broadcast_to([128, G * D2])
                )
                nc.sync.dma_start(x2b[:, :], src)
                x2b4 = x2b.rearrange("p (g d) -> p g d", d=D2).unsqueeze(1)

                diff = dpool.tile([128, UT, G, D2], dt, tag="diff", name="diff")
                if td > 0:
                    nc.vector.tensor_tensor(
                        diff[:, 0:td, :, :],
                        x2b4.broadcast_to([128, td, G, D2]),
                        w2c[:, 0:td, :].unsqueeze(2).broadcast_to([128, td, G, D2]),
                        mybir.AluOpType.subtract,
                    )
                if k_gp > 0:
                    nc.gpsimd.tensor_tensor(
                        diff[:, td:UT, :, :],
                        x2b4.broadcast_to([128, k_gp, G, D2]),
                        w2g[:, td:UT, :].unsqueeze(2).broadcast_to(
                            [128, k_gp, G, D2]
                        ),
                        mybir.AluOpType.subtract,
                    )

                width = D2
                osl = osb[:, :, c * G : (c + 1) * G]
                if USE_TREE:
                    while width > MINW_DVE and width % 2 == 0:
                        half = width // 2
                        nc.vector.tensor_tensor(
                            diff[:, :, :, 0:half],
                            diff[:, :, :, 0:half],
                            diff[:, :, :, half:width],
                            mybir.AluOpType.min,
                        )
                        width = half
                nc.vector.tensor_reduce(
                    osl,
                    diff[:, :, :, 0:width],
                    axis=mybir.AxisListType.X,
                    op=mybir.AluOpType.min,
                )

            for t in range(UT):
                nc.sync.dma_start(out[t, :, :], osb[:, t, :])

    nc.compile()
    return nc


def _prep_inputs(x, Wmin, Wmax):
    ndt = _np_dt()
    x2 = np.concatenate([x, -x], axis=1).astype(ndt)
    w2 = np.concatenate([Wmin, -Wmax], axis=1).astype(ndt)
    in_maps = []
    for r in range(N_CORES):
        in_maps.append(
            {
                "x2": np.ascontiguousarray(x2[r * QS : (r + 1) * QS]),
                "w2": np.ascontiguousarray(w2),
            }
        )
    return in_maps


def _assemble(results):
    ys = []
    for r in range(N_CORES):
        o = results[r]["out"]
        ys.append(o.reshape(UNITS, QS).T)
    return np.ascontiguousarray(np.concatenate(ys, axis=0).astype(np.float32))


_NC_CACHE = {}


def _get_nc():
    key = (DT_NAME, G, GP_SUBS, USE_TREE, MINW_DVE)
    if key not in _NC_CACHE:
        _NC_CACHE[key] = build_nc()
    return _NC_CACHE[key]


def run(x, Wmin, Wmax, trace=False):
    nc = _get_nc()
    in_maps = _prep_inputs(x, Wmin, Wmax)
    res = run_bass_kernel_spmd(nc, in_maps, core_ids=list(range(N_CORES)), trace=trace)
    return _assemble(res.results), res


def kernel(x, Wmin, Wmax):
    y, _ = run(x, Wmin, Wmax, trace=False)
    return y
